# revision 12
# baseline (speedup 1.0000x reference)
"""Causal temporal attention (CausalGroupNorm + per-pixel temporal attention)
on 8 Trainium2 NeuronCores.

Sharding: data-parallel over the B*H*W pixel pseudo-batch. Core i handles
batch b = i//2 and h-rows [16*(i%2), 16*(i%2)+16) -- 512 pixels per core,
each with a [C=512, T=32] temporal sequence.

Key algebraic restructure vs the straightforward kernel:
  S  = q^T k / sqrt(C) = h~^T M h~   with M = diag(g) Wq^T Wk diag(g) / sqrt(C)
  out = Wp (V P^T)      = N (h~ P^T)  with N = Wp Wv diag(g)
(h~ is the exactly-normalized input; M, N precomputed on host in fp64.)
This removes two of the four CxC GEMMs per pixel block and all the
rank-1 mean/bias correction machinery (h~ is exact).

Memory plan per core (SBUF 224 KiB/partition):
  - phase 1 streams x once from HBM; blocks 0-2 are retained as bf16
    (96 KiB/part), block 3 is retained as fp32 in the xy tile pool.
  - phase 3 never reloads x: h~ = xb*rstd - mean*rstd from SBUF, residual
    adds use the same retained tiles. Total HBM traffic = 1x read + 1x write.

Per-core pipeline (single Tile kernel):
  Phase 1: stream x tiles [128, 32t, 128w] fp32; bf16 copy (DVE),
           squares (DVE bf16 / Act fp32), colsum + colsum-of-squares on the
           PE (ones lhsT) into 4 persistent PSUM accumulators (partition
           bands = t-chunks); DVE reduce; DMA-gather stats to DRAM [1,64].
  Phase 2: pairwise AllReduce of stats, derive rstd | mean*rstd, broadcast
           to [128,64] via K=1 PE matmul (+ bf16 copy).
  Phase 3: per block (order 3,0,1,2), per h-chunk of 16 pixels:
           h~ (Pool mult + DVE subtract, bf16, pixel-major columns),
           f = M h~ (PE, Act evac), S = h~^T f + mask (PE), masked softmax
           (Act exp + fused rowsum, DVE recip/scale/transpose),
           hP = h~ P^T (PE, Act evac), out = N hP + residual (PE, DVE evac
           into fp32 y tiles), DMA store per block.
"""

import sys

sys.path.insert(0, "/opt/trn_rl_repo")

import numpy as np

C = 512
T = 32
HL = 16          # h-rows per core
W = 32
NL = HL * W      # pixels per core = 512
PB = 128         # pixels per block
NBLK = NL // PB  # 4 blocks
NPERSIST = 3     # blocks kept in bf16; the last block stays fp32 in xy pool
CC = C // 128    # 4 chunks
HCB = PB // 16   # h-chunks per block = 8
NELEM = float(C * 2 * NL)  # elements per (b,t) frame for the group norm
EPS = 1e-6

_CACHE = {}


def _build(collective=True, has_beta=False):
    from concourse import bacc, tile, mybir, bass

    f32 = mybir.dt.float32
    f32r = mybir.dt.float32r
    bf16 = mybir.dt.bfloat16
    Alu = mybir.AluOpType
    Act = mybir.ActivationFunctionType

    nc = bacc.Bacc("TRN2", target_bir_lowering=False, debug=False, num_devices=8)

    x_d = nc.dram_tensor("x", [C, T, HL, W], f32, kind="ExternalInput").ap()
    mt_d = nc.dram_tensor("mt", [C, C], bf16, kind="ExternalInput").ap()
    nt_d = nc.dram_tensor("nt", [C, C], bf16, kind="ExternalInput").ap()
    mask_d = nc.dram_tensor("mask", [128, 128], bf16, kind="ExternalInput").ap()
    ident_d = nc.dram_tensor("ident", [128, 128], bf16, kind="ExternalInput").ap()
    ones_d = nc.dram_tensor("ones", [128, 32], f32, kind="ExternalInput").ap()
    onesb_d = nc.dram_tensor("onesb", [128, 32], bf16, kind="ExternalInput").ap()
    bg_d = nc.dram_tensor("bg", [C], f32, kind="ExternalInput").ap()
    y_d = nc.dram_tensor("y", [C, T, HL, W], f32, kind="ExternalOutput").ap()

    xv = x_d.rearrange("c t h w -> c t (h w)")   # [512, 32, 512]
    yv = y_d.rearrange("c t h w -> c t (h w)")

    def ap3(base, off, dims):
        return bass.AP(tensor=base.tensor, offset=base.offset + off, ap=[base.ap[0]] + dims)

    with tile.TileContext(nc) as tc:
        from contextlib import ExitStack

        with ExitStack() as ctx:
            persist = ctx.enter_context(tc.tile_pool(name="persist", bufs=1))
            xypool = ctx.enter_context(tc.tile_pool(name="xy", bufs=4))
            dram = ctx.enter_context(tc.tile_pool(name="cdram", bufs=1, space="DRAM"))

            # ---- constants / weights
            mt_t = persist.tile([128, CC, C], bf16, tag="mt")
            nc.sync.dma_start(out=mt_t, in_=mt_d.rearrange("(cc p) o -> p cc o", p=128))
            nt_t = persist.tile([128, CC, C], bf16, tag="nt")
            nc.sync.dma_start(out=nt_t, in_=nt_d.rearrange("(cc p) o -> p cc o", p=128))
            mask_t = persist.tile([128, 128], bf16, tag="mask")
            nc.sync.dma_start(out=mask_t, in_=mask_d)
            ident_t = persist.tile([128, 128], bf16, tag="ident")
            nc.sync.dma_start(out=ident_t, in_=ident_d)
            ones_t = persist.tile([128, 32], f32r, tag="ones")
            nc.sync.dma_start(out=ones_t, in_=ones_d.bitcast(f32r))
            onesb_t = persist.tile([128, 32], bf16, tag="onesb")
            nc.sync.dma_start(out=onesb_t, in_=onesb_d)
            bg_t = persist.tile([128, CC], f32, tag="bg")
            nc.sync.dma_start(out=bg_t, in_=bg_d.rearrange("(cc p) -> p cc", p=128))

            ones1_t = persist.tile([1, 128], f32, tag="ones1")
            nc.vector.memset(ones1_t, 1.0)
            eps_t = persist.tile([1, 1], f32, tag="eps")
            nc.vector.memset(eps_t, EPS)

            # rstd | mean*rstd broadcast across partitions (fp32 + bf16)
            rmrb_t = persist.tile([128, 64], f32, tag="rmrb")
            rmrb_bf = persist.tile([128, 64], bf16, tag="rmrbb")

            # persistent bf16 copies of x, blocks 0..2
            xb = [[persist.tile([128, T, PB], bf16, tag=f"xb{b}_{c}", name=f"xb{b}_{c}")
                   for c in range(CC)] for b in range(NPERSIST)]
            x3 = []  # block-3 fp32 tiles (xy pool), written in phase 1

            st_in = dram.tile([1, 64], f32)
            st_out = dram.tile([1, 64], f32)

            # ================= Phase 1: stats (+ bf16 retention) ============
            with ExitStack() as p1:
                sb1 = p1.enter_context(tc.tile_pool(name="p1sb", bufs=1))
                ps1 = p1.enter_context(tc.tile_pool(name="p1ps", bufs=1, space="PSUM"))

                # 16 accumulators ([32, 512] each) for (kind: sum/sumsq) x
                # (slice s = t-chunk of 4): 3 partition bands {0,32,64} per
                # tile (HW forbids matmul dst partition 96), 3 tiles per kind.
                # accumulator (k, s) -> tile k*3 + s//3, band s%3.
                acc = [ps1.tile([128, 512], f32, tag=f"acc{k}", name=f"acc{k}")
                       for k in range(6)]

                for blk in range(NBLK):
                    for cc in range(CC):
                        first = blk == 0 and cc == 0
                        last = blk == NBLK - 1 and cc == CC - 1
                        if blk < NPERSIST:
                            halves = []
                            for hf in range(2):
                                x1 = sb1.tile([128, 16, PB], f32, tag="x1", bufs=3)
                                nc.sync.dma_start(
                                    out=x1,
                                    in_=xv[cc * 128:(cc + 1) * 128,
                                           16 * hf:16 * (hf + 1),
                                           PB * blk:PB * (blk + 1)],
                                )
                                nc.vector.tensor_copy(
                                    out=xb[blk][cc][:, 16 * hf:16 * (hf + 1), :],
                                    in_=x1)
                                sub = xb[blk][cc][:, 16 * hf:16 * (hf + 1), :]
                                halves.append((sub.rearrange("p t w -> p (t w)"),
                                               onesb_t, False))
                        else:
                            x3t = xypool.tile([128, T, PB], f32r, tag="xy",
                                              name=f"x3_{cc}")
                            nc.sync.dma_start(
                                out=x3t,
                                in_=xv[cc * 128:(cc + 1) * 128, :,
                                       PB * blk:PB * (blk + 1)].bitcast(f32r),
                            )
                            x3.append(x3t)
                            halves = []
                            for hf in range(2):
                                sub = x3t[:, 16 * hf:16 * (hf + 1), :]
                                halves.append((sub.rearrange("p t w -> p (t w)"),
                                               ones_t, True))
                        for hf, (src2, sum_lhsT, is_f32) in enumerate(halves):
                            sq = sb1.tile([128, 16 * PB], bf16, tag="sq", bufs=3)
                            if is_f32:
                                nc.scalar.activation(out=sq, in_=src2, func=Act.Square)
                                sum_rhs = src2
                            else:
                                nc.vector.tensor_tensor(out=sq, in0=src2, in1=src2,
                                                        op=Alu.mult)
                                sum_rhs = src2
                            for i in range(4):
                                s = 4 * hf + i
                                tj, b = s // 3, s % 3
                                sl = slice(512 * i, 512 * (i + 1))
                                nc.tensor.matmul(
                                    acc[tj][32 * b:32 * (b + 1), :],
                                    lhsT=sum_lhsT, rhs=sum_rhs[:, sl],
                                    start=first, stop=last,
                                    tile_position=(0, 32 * b))
                                nc.tensor.matmul(
                                    acc[3 + tj][32 * b:32 * (b + 1), :],
                                    lhsT=onesb_t, rhs=sq[:, sl],
                                    start=first, stop=last,
                                    tile_position=(0, 32 * b))

                # reduce over w: [128, 4t, 128w] -> [128, 4]; band b holds a
                # t-chunk. Then DMA-gather bands into st_in = [sum(32)|sq(32)].
                for k in range(2):
                    for tj in range(3):
                        nb = 3 if tj < 2 else 2
                        red = sb1.tile([128, 4], f32, tag="red", bufs=6)
                        nc.vector.reduce_sum(
                            out=red[0:32 * nb, :],
                            in_=acc[3 * k + tj][0:32 * nb, :].rearrange(
                                "p (t w) -> p t w", t=4),
                            axis=mybir.AxisListType.X,
                        )
                        gview = bass.AP(
                            tensor=red.tensor, offset=red.offset,
                            ap=[[red.ap[0][0] * 32, nb], [1, 4]])
                        off = 32 * k + 12 * tj
                        nc.gpsimd.dma_start(out=st_in[:, off:off + 4 * nb],
                                            in_=gview)

            # ================= Phase 2: allreduce + derive =================
            with ExitStack() as p2:
                sb2 = p2.enter_context(tc.tile_pool(name="p2sb", bufs=1))
                ps2 = p2.enter_context(tc.tile_pool(name="p2ps", bufs=1, space="PSUM"))

                if collective:
                    nc.gpsimd.collective_compute(
                        "AllReduce",
                        Alu.add,
                        replica_groups=[[0, 1], [2, 3], [4, 5], [6, 7]],
                        ins=[st_in.opt()],
                        outs=[st_out.opt()],
                    )
                else:
                    nc.gpsimd.dma_start(out=st_out, in_=st_in)
                vr_t = sb2.tile([1, 64], f32, tag="vr")
                nc.gpsimd.dma_start(out=vr_t, in_=st_out)

                mean_t = sb2.tile([1, 32], f32, tag="mean")
                nc.scalar.mul(out=mean_t, in_=vr_t[:, 0:32], mul=1.0 / NELEM)
                var_t = sb2.tile([1, 32], f32, tag="var")
                nc.scalar.mul(out=var_t, in_=vr_t[:, 32:64], mul=1.0 / NELEM)
                msq_t = sb2.tile([1, 32], f32, tag="msq")
                nc.vector.tensor_tensor(out=msq_t, in0=mean_t, in1=mean_t, op=Alu.mult)
                nc.vector.tensor_tensor(out=var_t, in0=var_t, in1=msq_t, op=Alu.subtract)
                nc.scalar.activation(out=var_t, in_=var_t, func=Act.Sqrt,
                                     bias=eps_t, scale=1.0)
                rm_t = sb2.tile([1, 64], f32, tag="rm")
                nc.vector.reciprocal(out=rm_t[:, 0:32], in_=var_t)
                nc.vector.tensor_tensor(out=rm_t[:, 32:64], in0=mean_t,
                                        in1=rm_t[:, 0:32], op=Alu.mult)
                bc_ps = ps2.tile([128, 64], f32, tag="bc")
                nc.tensor.matmul(bc_ps, lhsT=ones1_t, rhs=rm_t, start=True, stop=True)
                nc.vector.tensor_copy(out=rmrb_t, in_=bc_ps)
                nc.vector.tensor_copy(out=rmrb_bf, in_=bc_ps)

            # broadcast views: [128, 16 pix (stride 0), 32 t]
            rbv = ap3(rmrb_t, 0, [[0, 16], [1, 32]])
            mrv = ap3(rmrb_bf, 32, [[0, 16], [1, 32]])

            # ================= Phase 3: main =================
            with ExitStack() as p3:
                sb3 = p3.enter_context(tc.tile_pool(name="p3sb", bufs=1))
                mmps = p3.enter_context(tc.tile_pool(name="mmps", bufs=6, space="PSUM"))
                pops = p3.enter_context(tc.tile_pool(name="pops", bufs=2, space="PSUM"))

                for blk in [NBLK - 1] + list(range(NPERSIST)):
                    if blk == NBLK - 1:
                        cur = x3          # fp32, in-place residual
                        y_t = x3
                    else:
                        cur = xb[blk]     # bf16
                        y_t = [xypool.tile([128, T, PB], f32, tag="xy",
                                            name=f"y{blk}_{i}")
                               for i in range(CC)]

                    for hc in range(HCB):
                        # ---- h~ = x*r - m*r (pixel-major [128, 16p x 32t])
                        ht = []
                        for cc in range(CC):
                            h = sb3.tile([128, 512], bf16, tag="h", bufs=14)
                            h3 = h.rearrange("q (p t) -> q p t", p=16)
                            srcv = ap3(cur[cc], 16 * hc, [[1, 16], [PB, 32]])
                            nc.gpsimd.tensor_tensor(out=h3, in0=srcv, in1=rbv, op=Alu.mult)
                            if has_beta:
                                nc.vector.scalar_tensor_tensor(
                                    out=h3, in0=h3, scalar=bg_t[:, cc:cc + 1],
                                    in1=mrv, op0=Alu.add, op1=Alu.subtract)
                            else:
                                nc.vector.tensor_tensor(out=h3, in0=h3, in1=mrv,
                                                        op=Alu.subtract)
                            ht.append(h)

                        # ---- f = M h~
                        ft = []
                        for oc in range(CC):
                            fp = mmps.tile([128, 512], f32, tag="mm")
                            for cc in range(CC):
                                nc.tensor.matmul(
                                    fp,
                                    lhsT=mt_t[:, cc, 128 * oc:128 * (oc + 1)],
                                    rhs=ht[cc],
                                    start=(cc == 0),
                                    stop=(cc == CC - 1),
                                )
                            f_sb = sb3.tile([128, 512], bf16, tag="f", bufs=10)
                            nc.scalar.copy(out=f_sb, in_=fp)
                            ft.append(f_sb)

                        # ---- attention per 4-pixel group
                        hp_sb = sb3.tile([128, CC, 512], bf16, tag="hp", bufs=2)
                        for g in range(4):
                            sp = mmps.tile([128, 128], f32, tag="mm")
                            for cc in range(CC):
                                nc.tensor.matmul(
                                    sp,
                                    lhsT=ht[cc][:, 128 * g:128 * (g + 1)],
                                    rhs=ft[cc][:, 128 * g:128 * (g + 1)],
                                    start=(cc == 0),
                                    stop=False,
                                )
                            nc.tensor.matmul(
                                sp,
                                lhsT=ident_t,
                                rhs=mask_t,
                                start=False,
                                stop=True,
                            )
                            p_t = sb3.tile([128, 128], bf16, tag="p", bufs=8)
                            rs_t = sb3.tile([128, 1], f32, tag="rs", bufs=4)
                            nc.scalar.activation(
                                out=p_t,
                                in_=sp,
                                func=Act.Exp,
                                accum_out=rs_t)
                            ri_t = sb3.tile([128, 1], f32, tag="ri", bufs=4)
                            nc.vector.reciprocal(out=ri_t, in_=rs_t)
                            nc.vector.tensor_scalar_mul(out=p_t, in0=p_t, scalar1=ri_t)
                            pt_t = sb3.tile([128, 128], bf16, tag="pt", bufs=8)
                            nc.vector.transpose(out=pt_t, in_=p_t)
                            hp_ps = pops.tile([128, CC, 128], f32, tag="po")
                            for cc in range(CC):
                                nc.tensor.matmul(
                                    hp_ps[:, cc, :],
                                    lhsT=ht[cc][:, 128 * g:128 * (g + 1)],
                                    rhs=pt_t,
                                    start=True,
                                    stop=True,
                                )
                            nc.scalar.copy(
                                out=hp_sb[:, :, 128 * g:128 * (g + 1)],
                                in_=hp_ps)

                        # ---- out = N hP + residual
                        for oc in range(CC):
                            pp = mmps.tile([128, 512], f32, tag="mm")
                            for cc in range(CC):
                                nc.tensor.matmul(
                                    pp,
                                    lhsT=nt_t[:, cc, 128 * oc:128 * (oc + 1)],
                                    rhs=hp_sb[:, cc, :],
                                    start=(cc == 0),
                                    stop=(cc == CC - 1),
                                )
                            dstv = ap3(y_t[oc], 16 * hc, [[1, 16], [PB, 32]])
                            resv = ap3(cur[oc], 16 * hc, [[1, 16], [PB, 32]])
                            nc.vector.tensor_tensor(
                                out=dstv,
                                in0=pp.rearrange("q (p t) -> q p t", p=16),
                                in1=resv,
                                op=Alu.add,
                            )

                    # ---- store block
                    for cc in range(CC):
                        dst = yv[cc * 128:(cc + 1) * 128, :, PB * blk:PB * (blk + 1)]
                        if blk == NBLK - 1:
                            dst = dst.bitcast(f32r)
                        nc.sync.dma_start(out=dst, in_=y_t[cc])

    nc.compile()
    return nc


def _host_prep(gamma, beta, wq, wk, wv, wproj):
    import ml_dtypes

    scale = float(C) ** -0.5
    g64 = gamma.astype(np.float64)
    M = (np.diag(g64) @ wq.astype(np.float64).T @ wk.astype(np.float64)
         @ np.diag(g64)) * scale
    N = wproj.astype(np.float64) @ wv.astype(np.float64) @ np.diag(g64)
    mt = np.ascontiguousarray(M.T.astype(ml_dtypes.bfloat16))
    nt = np.ascontiguousarray(N.T.astype(ml_dtypes.bfloat16))
    assert np.all(gamma != 0), "gamma must be nonzero"
    bg = (beta.astype(np.float64) / g64).astype(np.float32)
    has_beta = bool(np.any(beta != 0))

    # additive causal/block-diag mask for [128 rows=(p,t), 128 cols=(p,s)]
    idx = np.arange(128)
    pi, ti = idx[:, None] // 32, idx[:, None] % 32
    pj, tj = idx[None, :] // 32, idx[None, :] % 32
    mask = np.where((pi == pj) & (tj <= ti), 0.0, -1e30).astype(ml_dtypes.bfloat16)
    ident = np.eye(128, dtype=ml_dtypes.bfloat16)
    ones = np.ones((128, 32), dtype=np.float32)
    onesb = np.ones((128, 32), dtype=ml_dtypes.bfloat16)
    return mt, nt, mask, ident, ones, onesb, bg, has_beta


def _get_runner(has_beta=False):
    """Build (once) a sharded jitted executable for the compiled Bass module."""
    key = ("runner", has_beta)
    if key in _CACHE:
        return _CACHE[key]

    import jax
    from jax.sharding import Mesh, PartitionSpec
    from jax.experimental.shard_map import shard_map
    from concourse import bass2jax, mybir

    nckey = ("nc", has_beta)
    nc = _CACHE.get(nckey)
    if nc is None:
        nc = _build(has_beta=has_beta)
        _CACHE[nckey] = nc

    bass2jax.install_neuronx_cc_hook()

    partition_name = nc.partition_id_tensor.name if nc.partition_id_tensor else None
    in_names, out_names, out_avals = [], [], []
    for alloc in nc.m.functions[0].allocations:
        if not isinstance(alloc, mybir.MemoryLocationSet):
            continue
        name = alloc.memorylocations[0].name
        if alloc.kind == "ExternalInput":
            if name != partition_name:
                in_names.append(name)
        elif alloc.kind == "ExternalOutput":
            out_names.append(name)
            shape = tuple(alloc.tensor_shape)
            dtype = mybir.dt.np(alloc.dtype)
            out_avals.append(jax.core.ShapedArray(shape, dtype))
    n_params = len(in_names)
    n_outs = len(out_avals)
    all_in_names = list(in_names) + list(out_names)
    if partition_name is not None:
        all_in_names.append(partition_name)
    donate = tuple(range(n_params, n_params + n_outs))

    def _body(*args):
        operands = list(args)
        if partition_name is not None:
            operands.append(bass2jax.partition_id_tensor())
        outs = bass2jax._bass_exec_p.bind(
            *operands,
            out_avals=tuple(out_avals),
            in_names=tuple(all_in_names),
            out_names=tuple(out_names),
            lowering_input_output_aliases=(),
            sim_require_finite=True,
            sim_require_nnan=True,
            nc=nc,
        )
        return tuple(outs)

    devices = jax.devices()[:8]
    mesh = Mesh(np.asarray(devices), ("core",))
    in_specs = (PartitionSpec("core"),) * (n_params + n_outs)
    out_specs = (PartitionSpec("core"),) * n_outs
    sharded = jax.jit(
        shard_map(_body, mesh=mesh, in_specs=in_specs, out_specs=out_specs,
                  check_rep=False),
        donate_argnums=donate,
        keep_unused=True,
    )
    runner = {
        "sharded": sharded,
        "mesh": mesh,
        "in_names": in_names,
        "out_names": out_names,
        "out_avals": out_avals,
    }
    _CACHE[key] = runner
    return runner


def _run(in_maps, has_beta=False):
    runner = _get_runner(has_beta)
    in_names = runner["in_names"]
    out_avals = runner["out_avals"]
    per_core = [[np.asarray(m[name]) for name in in_names] for m in in_maps]
    concat_in = [
        np.concatenate([per_core[c][i] for c in range(8)], axis=0)
        for i in range(len(in_names))
    ]
    concat_zeros = [
        np.zeros((8 * a.shape[0], *a.shape[1:]), a.dtype) for a in out_avals
    ]
    out_arrs = runner["sharded"](*concat_in, *concat_zeros)
    _CACHE["last_run"] = (concat_in, [tuple(a.shape) for a in out_avals])
    _CACHE["last_beta"] = has_beta
    return [
        {
            name: np.asarray(out_arrs[i]).reshape(8, *out_avals[i].shape)[c]
            for i, name in enumerate(runner["out_names"])
        }
        for c in range(8)
    ]


def bench(iters=10):
    """Time device-side execution with inputs pre-staged on the devices."""
    import time
    import jax
    from jax.sharding import NamedSharding, PartitionSpec

    runner = _get_runner(_CACHE.get("last_beta", False))
    concat_in, out_shapes = _CACHE["last_run"]
    sharding = NamedSharding(runner["mesh"], PartitionSpec("core"))
    dev_in = [jax.device_put(a, sharding) for a in concat_in]
    for a in dev_in:
        a.block_until_ready()
    times = []
    for _ in range(iters):
        zeros = [
            jax.device_put(np.zeros((8 * s[0], *s[1:]), np.float32), sharding)
            for s in out_shapes
        ]
        for z in zeros:
            z.block_until_ready()
        t0 = time.perf_counter()
        outs = runner["sharded"](*dev_in, *zeros)
        for o in outs:
            o.block_until_ready()
        t1 = time.perf_counter()
        times.append(t1 - t0)
    return times


def kernel(x, gamma, beta, wq, wk, wv, wproj):
    x = np.asarray(x, dtype=np.float32)
    gamma = np.asarray(gamma, dtype=np.float32)
    beta = np.asarray(beta, dtype=np.float32)
    wq = np.asarray(wq, dtype=np.float32)
    wk = np.asarray(wk, dtype=np.float32)
    wv = np.asarray(wv, dtype=np.float32)
    wproj = np.asarray(wproj, dtype=np.float32)

    mt, nt, mask, ident, ones, onesb, bg, has_beta = _host_prep(
        gamma, beta, wq, wk, wv, wproj)

    B = x.shape[0]
    in_maps = []
    for i in range(8):
        b, h0 = i // 2, HL * (i % 2)
        in_maps.append({
            "x": np.ascontiguousarray(x[b, :, :, h0:h0 + HL, :]),
            "mt": mt, "nt": nt, "mask": mask, "ident": ident,
            "ones": ones, "onesb": onesb, "bg": bg,
        })

    results = _run(in_maps, has_beta)

    y = np.empty((B, C, T, 2 * HL, W), dtype=np.float32)
    for i in range(8):
        b, h0 = i // 2, HL * (i % 2)
        y[b, :, :, h0:h0 + HL, :] = results[i]["y"]
    return y


# revision 13
# speedup vs baseline: 1.0316x; 1.0316x over previous
"""Causal temporal attention (CausalGroupNorm + per-pixel temporal attention)
on 8 Trainium2 NeuronCores.

Sharding: data-parallel over the B*H*W pixel pseudo-batch. Core i handles
batch b = i//2 and h-rows [16*(i%2), 16*(i%2)+16) -- 512 pixels per core,
each with a [C=512, T=32] temporal sequence.

Key algebraic restructure vs the straightforward kernel:
  S  = q^T k / sqrt(C) = h~^T M h~   with M = diag(g) Wq^T Wk diag(g) / sqrt(C)
  out = Wp (V P^T)      = N (h~ P^T)  with N = Wp Wv diag(g)
(h~ is the exactly-normalized input; M, N precomputed on host in fp64.)
This removes two of the four CxC GEMMs per pixel block and all the
rank-1 mean/bias correction machinery (h~ is exact).

Memory plan per core (SBUF 224 KiB/partition):
  - phase 1 streams x once from HBM; blocks 0-2 are retained as bf16
    (96 KiB/part), block 3 is retained as fp32 in the xy tile pool.
  - phase 3 never reloads x: h~ = xb*rstd - mean*rstd from SBUF, residual
    adds use the same retained tiles. Total HBM traffic = 1x read + 1x write.

Per-core pipeline (single Tile kernel):
  Phase 1: stream x tiles [128, 32t, 128w] fp32; bf16 copy (DVE),
           squares (DVE bf16 / Act fp32), colsum + colsum-of-squares on the
           PE (ones lhsT) into 4 persistent PSUM accumulators (partition
           bands = t-chunks); DVE reduce; DMA-gather stats to DRAM [1,64].
  Phase 2: pairwise AllReduce of stats, derive rstd | mean*rstd, broadcast
           to [128,64] via K=1 PE matmul (+ bf16 copy).
  Phase 3: per block (order 3,0,1,2), per h-chunk of 16 pixels:
           h~ (Pool mult + DVE subtract, bf16, pixel-major columns),
           f = M h~ (PE, Act evac), S = h~^T f + mask (PE), masked softmax
           (Act exp + fused rowsum, DVE recip/scale/transpose),
           hP = h~ P^T (PE, Act evac), out = N hP + residual (PE, DVE evac
           into fp32 y tiles), DMA store per block.
"""

import sys

sys.path.insert(0, "/opt/trn_rl_repo")

import numpy as np

C = 512
T = 32
HL = 16          # h-rows per core
W = 32
NL = HL * W      # pixels per core = 512
PB = 128         # pixels per block
NBLK = NL // PB  # 4 blocks
NPERSIST = 3     # blocks kept in bf16; the last block stays fp32 in xy pool
CC = C // 128    # 4 chunks
HCB = PB // 16   # h-chunks per block = 8
NELEM = float(C * 2 * NL)  # elements per (b,t) frame for the group norm
EPS = 1e-6

_CACHE = {}


def _build(collective=True, has_beta=False):
    from concourse import bacc, tile, mybir, bass

    f32 = mybir.dt.float32
    f32r = mybir.dt.float32r
    bf16 = mybir.dt.bfloat16
    Alu = mybir.AluOpType
    Act = mybir.ActivationFunctionType

    nc = bacc.Bacc("TRN2", target_bir_lowering=False, debug=False, num_devices=8)

    x_d = nc.dram_tensor("x", [C, T, HL, W], f32, kind="ExternalInput").ap()
    mt_d = nc.dram_tensor("mt", [C, C], bf16, kind="ExternalInput").ap()
    nt_d = nc.dram_tensor("nt", [C, C], bf16, kind="ExternalInput").ap()
    mask_d = nc.dram_tensor("mask", [128, 128], bf16, kind="ExternalInput").ap()
    ident_d = nc.dram_tensor("ident", [128, 128], bf16, kind="ExternalInput").ap()
    ones_d = nc.dram_tensor("ones", [128, 32], f32, kind="ExternalInput").ap()
    onesb_d = nc.dram_tensor("onesb", [128, 32], bf16, kind="ExternalInput").ap()
    bg_d = nc.dram_tensor("bg", [C], f32, kind="ExternalInput").ap()
    y_d = nc.dram_tensor("y", [C, T, HL, W], f32, kind="ExternalOutput").ap()

    xv = x_d.rearrange("c t h w -> c t (h w)")   # [512, 32, 512]
    yv = y_d.rearrange("c t h w -> c t (h w)")

    def ap3(base, off, dims):
        return bass.AP(tensor=base.tensor, offset=base.offset + off, ap=[base.ap[0]] + dims)

    with tile.TileContext(nc) as tc:
        from contextlib import ExitStack

        with ExitStack() as ctx:
            persist = ctx.enter_context(tc.tile_pool(name="persist", bufs=1))
            xypool = ctx.enter_context(tc.tile_pool(name="xy", bufs=4))
            dram = ctx.enter_context(tc.tile_pool(name="cdram", bufs=1, space="DRAM"))

            # ---- constants / weights
            mt_t = persist.tile([128, CC, C], bf16, tag="mt")
            nc.sync.dma_start(out=mt_t, in_=mt_d.rearrange("(cc p) o -> p cc o", p=128))
            nt_t = persist.tile([128, CC, C], bf16, tag="nt")
            nc.sync.dma_start(out=nt_t, in_=nt_d.rearrange("(cc p) o -> p cc o", p=128))
            mask_t = persist.tile([128, 128], bf16, tag="mask")
            nc.sync.dma_start(out=mask_t, in_=mask_d)
            ident_t = persist.tile([128, 128], bf16, tag="ident")
            nc.sync.dma_start(out=ident_t, in_=ident_d)
            ones_t = persist.tile([128, 32], f32r, tag="ones")
            nc.sync.dma_start(out=ones_t, in_=ones_d.bitcast(f32r))
            onesb_t = persist.tile([128, 32], bf16, tag="onesb")
            nc.sync.dma_start(out=onesb_t, in_=onesb_d)
            bg_t = persist.tile([128, CC], f32, tag="bg")
            nc.sync.dma_start(out=bg_t, in_=bg_d.rearrange("(cc p) -> p cc", p=128))

            ones1_t = persist.tile([1, 128], f32, tag="ones1")
            nc.vector.memset(ones1_t, 1.0)
            eps_t = persist.tile([1, 1], f32, tag="eps")
            nc.vector.memset(eps_t, EPS)

            # rstd | mean*rstd broadcast across partitions (fp32 + bf16)
            rmrb_t = persist.tile([128, 64], f32, tag="rmrb")
            rmrb_bf = persist.tile([128, 64], bf16, tag="rmrbb")

            # persistent bf16 copies of x, blocks 0..2
            xb = [[persist.tile([128, T, PB], bf16, tag=f"xb{b}_{c}", name=f"xb{b}_{c}")
                   for c in range(CC)] for b in range(NPERSIST)]
            x3 = []  # block-3 fp32 tiles (xy pool), written in phase 1

            st_in = dram.tile([1, 64], f32)
            st_out = dram.tile([1, 64], f32)

            # ================= Phase 1: stats (+ bf16 retention) ============
            with ExitStack() as p1:
                sb1 = p1.enter_context(tc.tile_pool(name="p1sb", bufs=1))
                ps1 = p1.enter_context(tc.tile_pool(name="p1ps", bufs=1, space="PSUM"))

                # 16 accumulators ([32, 512] each) for (kind: sum/sumsq) x
                # (slice s = t-chunk of 4): 3 partition bands {0,32,64} per
                # tile (HW forbids matmul dst partition 96), 3 tiles per kind.
                # accumulator (k, s) -> tile k*3 + s//3, band s%3.
                acc = [ps1.tile([128, 512], f32, tag=f"acc{k}", name=f"acc{k}")
                       for k in range(6)]

                for blk in range(NBLK):
                    for cc in range(CC):
                        first = blk == 0 and cc == 0
                        last = blk == NBLK - 1 and cc == CC - 1
                        if blk < NPERSIST:
                            halves = []
                            for hf in range(2):
                                x1 = sb1.tile([128, 16, PB], f32, tag="x1", bufs=3)
                                nc.sync.dma_start(
                                    out=x1,
                                    in_=xv[cc * 128:(cc + 1) * 128,
                                           16 * hf:16 * (hf + 1),
                                           PB * blk:PB * (blk + 1)],
                                )
                                nc.vector.tensor_copy(
                                    out=xb[blk][cc][:, 16 * hf:16 * (hf + 1), :],
                                    in_=x1)
                                sub = xb[blk][cc][:, 16 * hf:16 * (hf + 1), :]
                                halves.append((sub.rearrange("p t w -> p (t w)"),
                                               onesb_t, False))
                        else:
                            x3t = xypool.tile([128, T, PB], f32r, tag="xy",
                                              name=f"x3_{cc}")
                            nc.sync.dma_start(
                                out=x3t,
                                in_=xv[cc * 128:(cc + 1) * 128, :,
                                       PB * blk:PB * (blk + 1)].bitcast(f32r),
                            )
                            x3.append(x3t)
                            halves = []
                            for hf in range(2):
                                sub = x3t[:, 16 * hf:16 * (hf + 1), :]
                                halves.append((sub.rearrange("p t w -> p (t w)"),
                                               ones_t, True))
                        for hf, (src2, sum_lhsT, is_f32) in enumerate(halves):
                            sq = sb1.tile([128, 16 * PB], bf16, tag="sq", bufs=3)
                            if is_f32:
                                nc.scalar.activation(out=sq, in_=src2, func=Act.Square)
                                sum_rhs = src2
                            else:
                                nc.vector.tensor_tensor(out=sq, in0=src2, in1=src2,
                                                        op=Alu.mult)
                                sum_rhs = src2
                            for i in range(4):
                                s = 4 * hf + i
                                tj, b = s // 3, s % 3
                                sl = slice(512 * i, 512 * (i + 1))
                                nc.tensor.matmul(
                                    acc[tj][32 * b:32 * (b + 1), :],
                                    lhsT=sum_lhsT, rhs=sum_rhs[:, sl],
                                    start=first, stop=last,
                                    tile_position=(0, 32 * b))
                                nc.tensor.matmul(
                                    acc[3 + tj][32 * b:32 * (b + 1), :],
                                    lhsT=onesb_t, rhs=sq[:, sl],
                                    start=first, stop=last,
                                    tile_position=(0, 32 * b))

                # reduce over w: [128, 4t, 128w] -> [128, 4]; band b holds a
                # t-chunk. Then DMA-gather bands into st_in = [sum(32)|sq(32)].
                for k in range(2):
                    for tj in range(3):
                        nb = 3 if tj < 2 else 2
                        red = sb1.tile([128, 4], f32, tag="red", bufs=6)
                        nc.vector.reduce_sum(
                            out=red[0:32 * nb, :],
                            in_=acc[3 * k + tj][0:32 * nb, :].rearrange(
                                "p (t w) -> p t w", t=4),
                            axis=mybir.AxisListType.X,
                        )
                        gview = bass.AP(
                            tensor=red.tensor, offset=red.offset,
                            ap=[[red.ap[0][0] * 32, nb], [1, 4]])
                        off = 32 * k + 12 * tj
                        nc.gpsimd.dma_start(out=st_in[:, off:off + 4 * nb],
                                            in_=gview)

            # ================= Phase 2: allreduce + derive =================
            with ExitStack() as p2:
                sb2 = p2.enter_context(tc.tile_pool(name="p2sb", bufs=1))
                ps2 = p2.enter_context(tc.tile_pool(name="p2ps", bufs=1, space="PSUM"))

                if collective:
                    nc.gpsimd.collective_compute(
                        "AllReduce",
                        Alu.add,
                        replica_groups=[[0, 1], [2, 3], [4, 5], [6, 7]],
                        ins=[st_in.opt()],
                        outs=[st_out.opt()],
                    )
                else:
                    nc.gpsimd.dma_start(out=st_out, in_=st_in)
                vr_t = sb2.tile([1, 64], f32, tag="vr")
                nc.gpsimd.dma_start(out=vr_t, in_=st_out)

                mean_t = sb2.tile([1, 32], f32, tag="mean")
                nc.scalar.mul(out=mean_t, in_=vr_t[:, 0:32], mul=1.0 / NELEM)
                var_t = sb2.tile([1, 32], f32, tag="var")
                nc.scalar.mul(out=var_t, in_=vr_t[:, 32:64], mul=1.0 / NELEM)
                msq_t = sb2.tile([1, 32], f32, tag="msq")
                nc.vector.tensor_tensor(out=msq_t, in0=mean_t, in1=mean_t, op=Alu.mult)
                nc.vector.tensor_tensor(out=var_t, in0=var_t, in1=msq_t, op=Alu.subtract)
                nc.scalar.activation(out=var_t, in_=var_t, func=Act.Sqrt,
                                     bias=eps_t, scale=1.0)
                rm_t = sb2.tile([1, 64], f32, tag="rm")
                nc.vector.reciprocal(out=rm_t[:, 0:32], in_=var_t)
                nc.vector.tensor_tensor(out=rm_t[:, 32:64], in0=mean_t,
                                        in1=rm_t[:, 0:32], op=Alu.mult)
                bc_ps = ps2.tile([128, 64], f32, tag="bc")
                nc.tensor.matmul(bc_ps, lhsT=ones1_t, rhs=rm_t, start=True, stop=True)
                nc.vector.tensor_copy(out=rmrb_t, in_=bc_ps)
                nc.vector.tensor_copy(out=rmrb_bf, in_=bc_ps)

            # broadcast views: [128, 16 pix (stride 0), 32 t]
            rbv = ap3(rmrb_t, 0, [[0, 16], [1, 32]])
            mrv = ap3(rmrb_bf, 32, [[0, 16], [1, 32]])

            # ================= Phase 3: main =================
            with ExitStack() as p3:
                sb3 = p3.enter_context(tc.tile_pool(name="p3sb", bufs=1))
                fps_p = p3.enter_context(tc.tile_pool(name="fps", bufs=2, space="PSUM"))
                sps_p = p3.enter_context(tc.tile_pool(name="sps", bufs=2, space="PSUM"))
                pps_p = p3.enter_context(tc.tile_pool(name="pps", bufs=2, space="PSUM"))
                pops = p3.enter_context(tc.tile_pool(name="pops", bufs=2, space="PSUM"))

                for blk in [NBLK - 1] + list(range(NPERSIST)):
                    if blk == NBLK - 1:
                        cur = x3          # fp32, in-place residual
                        y_t = x3
                    else:
                        cur = xb[blk]     # bf16
                        y_t = [xypool.tile([128, T, PB], f32, tag="xy",
                                            name=f"y{blk}_{i}")
                               for i in range(CC)]

                    for hc in range(HCB):
                        # ---- h~ = x*r - m*r (pixel-major [128, 16p x 32t])
                        ht = []
                        for cc in range(CC):
                            h = sb3.tile([128, 512], bf16, tag="h", bufs=14)
                            h3 = h.rearrange("q (p t) -> q p t", p=16)
                            srcv = ap3(cur[cc], 16 * hc, [[1, 16], [PB, 32]])
                            nc.gpsimd.tensor_tensor(out=h3, in0=srcv, in1=rbv, op=Alu.mult)
                            if has_beta:
                                nc.vector.scalar_tensor_tensor(
                                    out=h3, in0=h3, scalar=bg_t[:, cc:cc + 1],
                                    in1=mrv, op0=Alu.add, op1=Alu.subtract)
                            else:
                                nc.vector.tensor_tensor(out=h3, in0=h3, in1=mrv,
                                                        op=Alu.subtract)
                            ht.append(h)

                        # ---- f = M h~
                        ft = []
                        for oc in range(CC):
                            fp = fps_p.tile([128, 512], f32, tag="fp")
                            for cc in range(CC):
                                nc.tensor.matmul(
                                    fp,
                                    lhsT=mt_t[:, cc, 128 * oc:128 * (oc + 1)],
                                    rhs=ht[cc],
                                    start=(cc == 0),
                                    stop=(cc == CC - 1),
                                )
                            f_sb = sb3.tile([128, 512], bf16, tag="f", bufs=10)
                            nc.scalar.copy(out=f_sb, in_=fp)
                            ft.append(f_sb)

                        # ---- attention per 4-pixel group
                        hp_sb = sb3.tile([128, CC, 512], bf16, tag="hp", bufs=2)
                        for g in range(4):
                            sp = sps_p.tile([128, 128], f32, tag="sp")
                            for cc in range(CC):
                                nc.tensor.matmul(
                                    sp,
                                    lhsT=ht[cc][:, 128 * g:128 * (g + 1)],
                                    rhs=ft[cc][:, 128 * g:128 * (g + 1)],
                                    start=(cc == 0),
                                    stop=False,
                                )
                            nc.tensor.matmul(
                                sp,
                                lhsT=ident_t,
                                rhs=mask_t,
                                start=False,
                                stop=True,
                            )
                            p_t = sb3.tile([128, 128], bf16, tag="p", bufs=8)
                            rs_t = sb3.tile([128, 1], f32, tag="rs", bufs=4)
                            nc.scalar.activation(
                                out=p_t,
                                in_=sp,
                                func=Act.Exp,
                                accum_out=rs_t)
                            ri_t = sb3.tile([128, 1], f32, tag="ri", bufs=4)
                            nc.vector.reciprocal(out=ri_t, in_=rs_t)
                            nc.vector.tensor_scalar_mul(out=p_t, in0=p_t, scalar1=ri_t)
                            pt_t = sb3.tile([128, 128], bf16, tag="pt", bufs=8)
                            nc.vector.transpose(out=pt_t, in_=p_t)
                            hp_ps = pops.tile([128, CC, 128], f32, tag="po")
                            for cc in range(CC):
                                nc.tensor.matmul(
                                    hp_ps[:, cc, :],
                                    lhsT=ht[cc][:, 128 * g:128 * (g + 1)],
                                    rhs=pt_t,
                                    start=True,
                                    stop=True,
                                )
                            nc.scalar.copy(
                                out=hp_sb[:, :, 128 * g:128 * (g + 1)],
                                in_=hp_ps)

                        # ---- out = N hP + residual
                        for oc in range(CC):
                            pp = pps_p.tile([128, 512], f32, tag="pp")
                            for cc in range(CC):
                                nc.tensor.matmul(
                                    pp,
                                    lhsT=nt_t[:, cc, 128 * oc:128 * (oc + 1)],
                                    rhs=hp_sb[:, cc, :],
                                    start=(cc == 0),
                                    stop=(cc == CC - 1),
                                )
                            dstv = ap3(y_t[oc], 16 * hc, [[1, 16], [PB, 32]])
                            resv = ap3(cur[oc], 16 * hc, [[1, 16], [PB, 32]])
                            nc.vector.tensor_tensor(
                                out=dstv,
                                in0=pp.rearrange("q (p t) -> q p t", p=16),
                                in1=resv,
                                op=Alu.add,
                            )

                    # ---- store block
                    for cc in range(CC):
                        dst = yv[cc * 128:(cc + 1) * 128, :, PB * blk:PB * (blk + 1)]
                        if blk == NBLK - 1:
                            dst = dst.bitcast(f32r)
                        nc.sync.dma_start(out=dst, in_=y_t[cc])

    nc.compile()
    return nc


def _host_prep(gamma, beta, wq, wk, wv, wproj):
    import ml_dtypes

    scale = float(C) ** -0.5
    g64 = gamma.astype(np.float64)
    M = (np.diag(g64) @ wq.astype(np.float64).T @ wk.astype(np.float64)
         @ np.diag(g64)) * scale
    N = wproj.astype(np.float64) @ wv.astype(np.float64) @ np.diag(g64)
    mt = np.ascontiguousarray(M.T.astype(ml_dtypes.bfloat16))
    nt = np.ascontiguousarray(N.T.astype(ml_dtypes.bfloat16))
    assert np.all(gamma != 0), "gamma must be nonzero"
    bg = (beta.astype(np.float64) / g64).astype(np.float32)
    has_beta = bool(np.any(beta != 0))

    # additive causal/block-diag mask for [128 rows=(p,t), 128 cols=(p,s)]
    idx = np.arange(128)
    pi, ti = idx[:, None] // 32, idx[:, None] % 32
    pj, tj = idx[None, :] // 32, idx[None, :] % 32
    mask = np.where((pi == pj) & (tj <= ti), 0.0, -1e30).astype(ml_dtypes.bfloat16)
    ident = np.eye(128, dtype=ml_dtypes.bfloat16)
    ones = np.ones((128, 32), dtype=np.float32)
    onesb = np.ones((128, 32), dtype=ml_dtypes.bfloat16)
    return mt, nt, mask, ident, ones, onesb, bg, has_beta


def _get_runner(has_beta=False):
    """Build (once) a sharded jitted executable for the compiled Bass module."""
    key = ("runner", has_beta)
    if key in _CACHE:
        return _CACHE[key]

    import jax
    from jax.sharding import Mesh, PartitionSpec
    from jax.experimental.shard_map import shard_map
    from concourse import bass2jax, mybir

    nckey = ("nc", has_beta)
    nc = _CACHE.get(nckey)
    if nc is None:
        nc = _build(has_beta=has_beta)
        _CACHE[nckey] = nc

    bass2jax.install_neuronx_cc_hook()

    partition_name = nc.partition_id_tensor.name if nc.partition_id_tensor else None
    in_names, out_names, out_avals = [], [], []
    for alloc in nc.m.functions[0].allocations:
        if not isinstance(alloc, mybir.MemoryLocationSet):
            continue
        name = alloc.memorylocations[0].name
        if alloc.kind == "ExternalInput":
            if name != partition_name:
                in_names.append(name)
        elif alloc.kind == "ExternalOutput":
            out_names.append(name)
            shape = tuple(alloc.tensor_shape)
            dtype = mybir.dt.np(alloc.dtype)
            out_avals.append(jax.core.ShapedArray(shape, dtype))
    n_params = len(in_names)
    n_outs = len(out_avals)
    all_in_names = list(in_names) + list(out_names)
    if partition_name is not None:
        all_in_names.append(partition_name)
    donate = tuple(range(n_params, n_params + n_outs))

    def _body(*args):
        operands = list(args)
        if partition_name is not None:
            operands.append(bass2jax.partition_id_tensor())
        outs = bass2jax._bass_exec_p.bind(
            *operands,
            out_avals=tuple(out_avals),
            in_names=tuple(all_in_names),
            out_names=tuple(out_names),
            lowering_input_output_aliases=(),
            sim_require_finite=True,
            sim_require_nnan=True,
            nc=nc,
        )
        return tuple(outs)

    devices = jax.devices()[:8]
    mesh = Mesh(np.asarray(devices), ("core",))
    in_specs = (PartitionSpec("core"),) * (n_params + n_outs)
    out_specs = (PartitionSpec("core"),) * n_outs
    sharded = jax.jit(
        shard_map(_body, mesh=mesh, in_specs=in_specs, out_specs=out_specs,
                  check_rep=False),
        donate_argnums=donate,
        keep_unused=True,
    )
    runner = {
        "sharded": sharded,
        "mesh": mesh,
        "in_names": in_names,
        "out_names": out_names,
        "out_avals": out_avals,
    }
    _CACHE[key] = runner
    return runner


def _run(in_maps, has_beta=False):
    runner = _get_runner(has_beta)
    in_names = runner["in_names"]
    out_avals = runner["out_avals"]
    per_core = [[np.asarray(m[name]) for name in in_names] for m in in_maps]
    concat_in = [
        np.concatenate([per_core[c][i] for c in range(8)], axis=0)
        for i in range(len(in_names))
    ]
    concat_zeros = [
        np.zeros((8 * a.shape[0], *a.shape[1:]), a.dtype) for a in out_avals
    ]
    out_arrs = runner["sharded"](*concat_in, *concat_zeros)
    _CACHE["last_run"] = (concat_in, [tuple(a.shape) for a in out_avals])
    _CACHE["last_beta"] = has_beta
    return [
        {
            name: np.asarray(out_arrs[i]).reshape(8, *out_avals[i].shape)[c]
            for i, name in enumerate(runner["out_names"])
        }
        for c in range(8)
    ]


def bench(iters=10):
    """Time device-side execution with inputs pre-staged on the devices."""
    import time
    import jax
    from jax.sharding import NamedSharding, PartitionSpec

    runner = _get_runner(_CACHE.get("last_beta", False))
    concat_in, out_shapes = _CACHE["last_run"]
    sharding = NamedSharding(runner["mesh"], PartitionSpec("core"))
    dev_in = [jax.device_put(a, sharding) for a in concat_in]
    for a in dev_in:
        a.block_until_ready()
    times = []
    for _ in range(iters):
        zeros = [
            jax.device_put(np.zeros((8 * s[0], *s[1:]), np.float32), sharding)
            for s in out_shapes
        ]
        for z in zeros:
            z.block_until_ready()
        t0 = time.perf_counter()
        outs = runner["sharded"](*dev_in, *zeros)
        for o in outs:
            o.block_until_ready()
        t1 = time.perf_counter()
        times.append(t1 - t0)
    return times


def kernel(x, gamma, beta, wq, wk, wv, wproj):
    x = np.asarray(x, dtype=np.float32)
    gamma = np.asarray(gamma, dtype=np.float32)
    beta = np.asarray(beta, dtype=np.float32)
    wq = np.asarray(wq, dtype=np.float32)
    wk = np.asarray(wk, dtype=np.float32)
    wv = np.asarray(wv, dtype=np.float32)
    wproj = np.asarray(wproj, dtype=np.float32)

    mt, nt, mask, ident, ones, onesb, bg, has_beta = _host_prep(
        gamma, beta, wq, wk, wv, wproj)

    B = x.shape[0]
    in_maps = []
    for i in range(8):
        b, h0 = i // 2, HL * (i % 2)
        in_maps.append({
            "x": np.ascontiguousarray(x[b, :, :, h0:h0 + HL, :]),
            "mt": mt, "nt": nt, "mask": mask, "ident": ident,
            "ones": ones, "onesb": onesb, "bg": bg,
        })

    results = _run(in_maps, has_beta)

    y = np.empty((B, C, T, 2 * HL, W), dtype=np.float32)
    for i in range(8):
        b, h0 = i // 2, HL * (i % 2)
        y[b, :, :, h0:h0 + HL, :] = results[i]["y"]
    return y


# revision 15
# speedup vs baseline: 1.0994x; 1.0658x over previous
"""Causal temporal attention (CausalGroupNorm + per-pixel temporal attention)
on 8 Trainium2 NeuronCores.

Sharding: data-parallel over the B*H*W pixel pseudo-batch. Core i handles
batch b = i//2 and h-rows [16*(i%2), 16*(i%2)+16) -- 512 pixels per core,
each with a [C=512, T=32] temporal sequence.

Key algebraic restructure vs the straightforward kernel:
  S  = q^T k / sqrt(C) = h~^T M h~   with M = diag(g) Wq^T Wk diag(g) / sqrt(C)
  out = Wp (V P^T)      = N (h~ P^T)  with N = Wp Wv diag(g)
(h~ is the exactly-normalized input; M, N precomputed on host in fp64.)
This removes two of the four CxC GEMMs per pixel block and all the
rank-1 mean/bias correction machinery (h~ is exact).

Memory plan per core (SBUF 224 KiB/partition):
  - phase 1 streams x once from HBM; blocks 0-2 are retained as bf16
    (96 KiB/part), block 3 is retained as fp32 in the xy tile pool.
  - phase 3 never reloads x: h~ = xb*rstd - mean*rstd from SBUF, residual
    adds use the same retained tiles. Total HBM traffic = 1x read + 1x write.

Per-core pipeline (single Tile kernel):
  Phase 1: stream x tiles [128, 32t, 128w] fp32; bf16 copy (DVE),
           squares (DVE bf16 / Act fp32), colsum + colsum-of-squares on the
           PE (ones lhsT) into 4 persistent PSUM accumulators (partition
           bands = t-chunks); DVE reduce; DMA-gather stats to DRAM [1,64].
  Phase 2: pairwise AllReduce of stats, derive rstd | mean*rstd, broadcast
           to [128,64] via K=1 PE matmul (+ bf16 copy).
  Phase 3: per block (order 3,0,1,2), per h-chunk of 16 pixels:
           h~ (Pool mult + DVE subtract, bf16, pixel-major columns),
           f = M h~ (PE, Act evac), S = h~^T f + mask (PE), masked softmax
           (Act exp + fused rowsum, DVE recip/scale/transpose),
           hP = h~ P^T (PE, Act evac), out = N hP + residual (PE, DVE evac
           into fp32 y tiles), DMA store per block.
"""

import sys

sys.path.insert(0, "/opt/trn_rl_repo")

import numpy as np

C = 512
T = 32
HL = 16          # h-rows per core
W = 32
NL = HL * W      # pixels per core = 512
PB = 128         # pixels per block
NBLK = NL // PB  # 4 blocks
NPERSIST = 3     # blocks kept in bf16; the last block stays fp32 in xy pool
CC = C // 128    # 4 chunks
HCB = PB // 16   # h-chunks per block = 8
NELEM = float(C * 2 * NL)  # elements per (b,t) frame for the group norm
EPS = 1e-6

_CACHE = {}


def _build(collective=True, has_beta=False):
    from concourse import bacc, tile, mybir, bass

    f32 = mybir.dt.float32
    f32r = mybir.dt.float32r
    bf16 = mybir.dt.bfloat16
    Alu = mybir.AluOpType
    Act = mybir.ActivationFunctionType

    nc = bacc.Bacc("TRN2", target_bir_lowering=False, debug=False, num_devices=8)

    x_d = nc.dram_tensor("x", [C, T, HL, W], f32, kind="ExternalInput").ap()
    mt_d = nc.dram_tensor("mt", [C, C], bf16, kind="ExternalInput").ap()
    nt_d = nc.dram_tensor("nt", [C, C], bf16, kind="ExternalInput").ap()
    mask_d = nc.dram_tensor("mask", [128, 128], bf16, kind="ExternalInput").ap()
    ident_d = nc.dram_tensor("ident", [128, 128], bf16, kind="ExternalInput").ap()
    ones_d = nc.dram_tensor("ones", [128, 32], f32, kind="ExternalInput").ap()
    onesb_d = nc.dram_tensor("onesb", [128, 32], bf16, kind="ExternalInput").ap()
    bg_d = nc.dram_tensor("bg", [C], f32, kind="ExternalInput").ap()
    y_d = nc.dram_tensor("y", [C, T, HL, W], f32, kind="ExternalOutput").ap()

    xv = x_d.rearrange("c t h w -> c t (h w)")   # [512, 32, 512]
    yv = y_d.rearrange("c t h w -> c t (h w)")

    def ap3(base, off, dims):
        return bass.AP(tensor=base.tensor, offset=base.offset + off, ap=[base.ap[0]] + dims)

    with tile.TileContext(nc) as tc:
        from contextlib import ExitStack

        with ExitStack() as ctx:
            persist = ctx.enter_context(tc.tile_pool(name="persist", bufs=1))
            xypool = ctx.enter_context(tc.tile_pool(name="xy", bufs=4))
            dram = ctx.enter_context(tc.tile_pool(name="cdram", bufs=1, space="DRAM"))

            # ---- constants / weights
            mt_t = persist.tile([128, CC, C], bf16, tag="mt")
            nc.sync.dma_start(out=mt_t, in_=mt_d.rearrange("(cc p) o -> p cc o", p=128))
            nt_t = persist.tile([128, CC, C], bf16, tag="nt")
            nc.sync.dma_start(out=nt_t, in_=nt_d.rearrange("(cc p) o -> p cc o", p=128))
            mask_t = persist.tile([128, 128], bf16, tag="mask")
            nc.sync.dma_start(out=mask_t, in_=mask_d)
            ident_t = persist.tile([128, 128], bf16, tag="ident")
            nc.sync.dma_start(out=ident_t, in_=ident_d)
            ones_t = persist.tile([128, 32], f32r, tag="ones")
            nc.sync.dma_start(out=ones_t, in_=ones_d.bitcast(f32r))
            onesb_t = persist.tile([128, 32], bf16, tag="onesb")
            nc.sync.dma_start(out=onesb_t, in_=onesb_d)
            bg_t = persist.tile([128, CC], f32, tag="bg")
            nc.sync.dma_start(out=bg_t, in_=bg_d.rearrange("(cc p) -> p cc", p=128))

            ones1_t = persist.tile([1, 128], f32, tag="ones1")
            nc.vector.memset(ones1_t, 1.0)
            eps_t = persist.tile([1, 1], f32, tag="eps")
            nc.vector.memset(eps_t, EPS)

            # rstd | mean*rstd broadcast across partitions (fp32 + bf16)
            rmrb_t = persist.tile([128, 64], f32, tag="rmrb")
            rmrb_bf = persist.tile([128, 64], bf16, tag="rmrbb")

            # persistent bf16 copies of x, blocks 0..2
            xb = [[persist.tile([128, T, PB], bf16, tag=f"xb{b}_{c}", name=f"xb{b}_{c}")
                   for c in range(CC)] for b in range(NPERSIST)]
            x3 = []  # block-3 fp32 tiles (xy pool), written in phase 1

            st_in = dram.tile([1, 64], f32)
            st_out = dram.tile([1, 64], f32)

            # ================= Phase 1: stats (+ bf16 retention) ============
            with ExitStack() as p1:
                sb1 = p1.enter_context(tc.tile_pool(name="p1sb", bufs=1))
                ps1 = p1.enter_context(tc.tile_pool(name="p1ps", bufs=1, space="PSUM"))

                # 16 accumulators ([32, 512] each) for (kind: sum/sumsq) x
                # (slice s = t-chunk of 4): 3 partition bands {0,32,64} per
                # tile (HW forbids matmul dst partition 96), 3 tiles per kind.
                # accumulator (k, s) -> tile k*3 + s//3, band s%3.
                acc = [ps1.tile([128, 512], f32, tag=f"acc{k}", name=f"acc{k}")
                       for k in range(6)]

                for blk in range(NBLK):
                    for cc in range(CC):
                        first = blk == 0 and cc == 0
                        last = blk == NBLK - 1 and cc == CC - 1
                        if blk < NPERSIST:
                            halves = []
                            for hf in range(2):
                                x1 = sb1.tile([128, 16, PB], f32, tag="x1", bufs=3)
                                nc.sync.dma_start(
                                    out=x1,
                                    in_=xv[cc * 128:(cc + 1) * 128,
                                           16 * hf:16 * (hf + 1),
                                           PB * blk:PB * (blk + 1)],
                                )
                                nc.vector.tensor_copy(
                                    out=xb[blk][cc][:, 16 * hf:16 * (hf + 1), :],
                                    in_=x1)
                                sub = xb[blk][cc][:, 16 * hf:16 * (hf + 1), :]
                                halves.append((sub.rearrange("p t w -> p (t w)"),
                                               onesb_t, False))
                        else:
                            x3t = xypool.tile([128, T, PB], f32r, tag="xy",
                                              name=f"x3_{cc}")
                            nc.sync.dma_start(
                                out=x3t,
                                in_=xv[cc * 128:(cc + 1) * 128, :,
                                       PB * blk:PB * (blk + 1)].bitcast(f32r),
                            )
                            x3.append(x3t)
                            halves = []
                            for hf in range(2):
                                sub = x3t[:, 16 * hf:16 * (hf + 1), :]
                                halves.append((sub.rearrange("p t w -> p (t w)"),
                                               ones_t, True))
                        for hf, (src2, sum_lhsT, is_f32) in enumerate(halves):
                            sq = sb1.tile([128, 16 * PB], bf16, tag="sq", bufs=3)
                            if is_f32:
                                nc.scalar.activation(out=sq, in_=src2, func=Act.Square)
                                sum_rhs = src2
                            else:
                                nc.vector.tensor_tensor(out=sq, in0=src2, in1=src2,
                                                        op=Alu.mult)
                                sum_rhs = src2
                            for i in range(4):
                                s = 4 * hf + i
                                tj, b = s // 3, s % 3
                                sl = slice(512 * i, 512 * (i + 1))
                                nc.tensor.matmul(
                                    acc[tj][32 * b:32 * (b + 1), :],
                                    lhsT=sum_lhsT, rhs=sum_rhs[:, sl],
                                    start=first, stop=last,
                                    tile_position=(0, 32 * b))
                                nc.tensor.matmul(
                                    acc[3 + tj][32 * b:32 * (b + 1), :],
                                    lhsT=onesb_t, rhs=sq[:, sl],
                                    start=first, stop=last,
                                    tile_position=(0, 32 * b))

                # reduce over w: [128, 4t, 128w] -> [128, 4]; band b holds a
                # t-chunk. Then DMA-gather bands into st_in = [sum(32)|sq(32)].
                for k in range(2):
                    for tj in range(3):
                        nb = 3 if tj < 2 else 2
                        red = sb1.tile([128, 4], f32, tag="red", bufs=6)
                        nc.vector.reduce_sum(
                            out=red[0:32 * nb, :],
                            in_=acc[3 * k + tj][0:32 * nb, :].rearrange(
                                "p (t w) -> p t w", t=4),
                            axis=mybir.AxisListType.X,
                        )
                        gview = bass.AP(
                            tensor=red.tensor, offset=red.offset,
                            ap=[[red.ap[0][0] * 32, nb], [1, 4]])
                        off = 32 * k + 12 * tj
                        nc.gpsimd.dma_start(out=st_in[:, off:off + 4 * nb],
                                            in_=gview)

            # ================= Phase 2: allreduce + derive =================
            with ExitStack() as p2:
                sb2 = p2.enter_context(tc.tile_pool(name="p2sb", bufs=1))
                ps2 = p2.enter_context(tc.tile_pool(name="p2ps", bufs=1, space="PSUM"))

                if collective:
                    nc.gpsimd.collective_compute(
                        "AllReduce",
                        Alu.add,
                        replica_groups=[[0, 1], [2, 3], [4, 5], [6, 7]],
                        ins=[st_in.opt()],
                        outs=[st_out.opt()],
                    )
                else:
                    nc.gpsimd.dma_start(out=st_out, in_=st_in)
                vr_t = sb2.tile([1, 64], f32, tag="vr")
                nc.gpsimd.dma_start(out=vr_t, in_=st_out)

                mean_t = sb2.tile([1, 32], f32, tag="mean")
                nc.scalar.mul(out=mean_t, in_=vr_t[:, 0:32], mul=1.0 / NELEM)
                var_t = sb2.tile([1, 32], f32, tag="var")
                nc.scalar.mul(out=var_t, in_=vr_t[:, 32:64], mul=1.0 / NELEM)
                msq_t = sb2.tile([1, 32], f32, tag="msq")
                nc.vector.tensor_tensor(out=msq_t, in0=mean_t, in1=mean_t, op=Alu.mult)
                nc.vector.tensor_tensor(out=var_t, in0=var_t, in1=msq_t, op=Alu.subtract)
                nc.scalar.activation(out=var_t, in_=var_t, func=Act.Sqrt,
                                     bias=eps_t, scale=1.0)
                rm_t = sb2.tile([1, 64], f32, tag="rm")
                nc.vector.reciprocal(out=rm_t[:, 0:32], in_=var_t)
                nc.vector.tensor_tensor(out=rm_t[:, 32:64], in0=mean_t,
                                        in1=rm_t[:, 0:32], op=Alu.mult)
                bc_ps = ps2.tile([128, 64], f32, tag="bc")
                nc.tensor.matmul(bc_ps, lhsT=ones1_t, rhs=rm_t, start=True, stop=True)
                nc.vector.tensor_copy(out=rmrb_t, in_=bc_ps)
                nc.vector.tensor_copy(out=rmrb_bf, in_=bc_ps)

            # broadcast views: [128, 16 pix (stride 0), 32 t]
            rbv = ap3(rmrb_t, 0, [[0, 16], [1, 32]])
            mrv = ap3(rmrb_bf, 32, [[0, 16], [1, 32]])

            # ================= Phase 3: main =================
            with ExitStack() as p3:
                sb3 = p3.enter_context(tc.tile_pool(name="p3sb", bufs=1))
                fps_p = p3.enter_context(tc.tile_pool(name="fps", bufs=3, space="PSUM"))
                sps_p = p3.enter_context(tc.tile_pool(name="sps", bufs=1, space="PSUM"))
                pps_p = p3.enter_context(tc.tile_pool(name="pps", bufs=3, space="PSUM"))
                pops = p3.enter_context(tc.tile_pool(name="pops", bufs=1, space="PSUM"))

                for blk in [NBLK - 1] + list(range(NPERSIST)):
                    if blk == NBLK - 1:
                        cur = x3          # fp32, in-place residual
                        y_t = x3
                    else:
                        cur = xb[blk]     # bf16
                        y_t = [xypool.tile([128, T, PB], f32, tag="xy",
                                            name=f"y{blk}_{i}")
                               for i in range(CC)]

                    for hc in range(HCB):
                        # ---- h~ = x*r - m*r (pixel-major [128, 16p x 32t])
                        ht = []
                        for cc in range(CC):
                            h = sb3.tile([128, 512], bf16, tag="h", bufs=14)
                            h3 = h.rearrange("q (p t) -> q p t", p=16)
                            srcv = ap3(cur[cc], 16 * hc, [[1, 16], [PB, 32]])
                            nc.gpsimd.tensor_tensor(out=h3, in0=srcv, in1=rbv, op=Alu.mult)
                            if has_beta:
                                nc.vector.scalar_tensor_tensor(
                                    out=h3, in0=h3, scalar=bg_t[:, cc:cc + 1],
                                    in1=mrv, op0=Alu.add, op1=Alu.subtract)
                            else:
                                nc.vector.tensor_tensor(out=h3, in0=h3, in1=mrv,
                                                        op=Alu.subtract)
                            ht.append(h)

                        # ---- f = M h~
                        ft = []
                        for oc in range(CC):
                            fp = fps_p.tile([128, 512], f32, tag="fp")
                            for cc in range(CC):
                                nc.tensor.matmul(
                                    fp,
                                    lhsT=mt_t[:, cc, 128 * oc:128 * (oc + 1)],
                                    rhs=ht[cc],
                                    start=(cc == 0),
                                    stop=(cc == CC - 1),
                                )
                            f_sb = sb3.tile([128, 512], bf16, tag="f", bufs=10)
                            nc.scalar.copy(out=f_sb, in_=fp)
                            ft.append(f_sb)

                        # ---- attention per 4-pixel group
                        hp_sb = sb3.tile([128, CC, 512], bf16, tag="hp", bufs=2)
                        for g in range(4):
                            sp = sps_p.tile([128, 128], f32, tag="sp")
                            for cc in range(CC):
                                nc.tensor.matmul(
                                    sp,
                                    lhsT=ht[cc][:, 128 * g:128 * (g + 1)],
                                    rhs=ft[cc][:, 128 * g:128 * (g + 1)],
                                    start=(cc == 0),
                                    stop=False,
                                )
                            nc.tensor.matmul(
                                sp,
                                lhsT=ident_t,
                                rhs=mask_t,
                                start=False,
                                stop=True,
                            )
                            p_t = sb3.tile([128, 128], bf16, tag="p", bufs=8)
                            rs_t = sb3.tile([128, 1], f32, tag="rs", bufs=4)
                            nc.scalar.activation(
                                out=p_t,
                                in_=sp,
                                func=Act.Exp,
                                accum_out=rs_t)
                            ri_t = sb3.tile([128, 1], f32, tag="ri", bufs=4)
                            nc.vector.reciprocal(out=ri_t, in_=rs_t)
                            nc.vector.tensor_scalar_mul(out=p_t, in0=p_t, scalar1=ri_t)
                            pt_t = sb3.tile([128, 128], bf16, tag="pt", bufs=8)
                            nc.vector.transpose(out=pt_t, in_=p_t)
                            hp_ps = pops.tile([128, CC, 128], f32, tag="po")
                            for cc in range(CC):
                                nc.tensor.matmul(
                                    hp_ps[:, cc, :],
                                    lhsT=ht[cc][:, 128 * g:128 * (g + 1)],
                                    rhs=pt_t,
                                    start=True,
                                    stop=True,
                                )
                            nc.scalar.copy(
                                out=hp_sb[:, :, 128 * g:128 * (g + 1)],
                                in_=hp_ps)

                        # ---- out = N hP + residual
                        for oc in range(CC):
                            pp = pps_p.tile([128, 512], f32, tag="pp")
                            for cc in range(CC):
                                nc.tensor.matmul(
                                    pp,
                                    lhsT=nt_t[:, cc, 128 * oc:128 * (oc + 1)],
                                    rhs=hp_sb[:, cc, :],
                                    start=(cc == 0),
                                    stop=(cc == CC - 1),
                                )
                            dstv = ap3(y_t[oc], 16 * hc, [[1, 16], [PB, 32]])
                            resv = ap3(cur[oc], 16 * hc, [[1, 16], [PB, 32]])
                            nc.vector.tensor_tensor(
                                out=dstv,
                                in0=pp.rearrange("q (p t) -> q p t", p=16),
                                in1=resv,
                                op=Alu.add,
                            )

                    # ---- store block
                    for cc in range(CC):
                        dst = yv[cc * 128:(cc + 1) * 128, :, PB * blk:PB * (blk + 1)]
                        if blk == NBLK - 1:
                            dst = dst.bitcast(f32r)
                        nc.sync.dma_start(out=dst, in_=y_t[cc])

    nc.compile()
    return nc


def _host_prep(gamma, beta, wq, wk, wv, wproj):
    import ml_dtypes

    scale = float(C) ** -0.5
    g64 = gamma.astype(np.float64)
    M = (np.diag(g64) @ wq.astype(np.float64).T @ wk.astype(np.float64)
         @ np.diag(g64)) * scale
    N = wproj.astype(np.float64) @ wv.astype(np.float64) @ np.diag(g64)
    mt = np.ascontiguousarray(M.T.astype(ml_dtypes.bfloat16))
    nt = np.ascontiguousarray(N.T.astype(ml_dtypes.bfloat16))
    assert np.all(gamma != 0), "gamma must be nonzero"
    bg = (beta.astype(np.float64) / g64).astype(np.float32)
    has_beta = bool(np.any(beta != 0))

    # additive causal/block-diag mask for [128 rows=(p,t), 128 cols=(p,s)]
    idx = np.arange(128)
    pi, ti = idx[:, None] // 32, idx[:, None] % 32
    pj, tj = idx[None, :] // 32, idx[None, :] % 32
    mask = np.where((pi == pj) & (tj <= ti), 0.0, -1e30).astype(ml_dtypes.bfloat16)
    ident = np.eye(128, dtype=ml_dtypes.bfloat16)
    ones = np.ones((128, 32), dtype=np.float32)
    onesb = np.ones((128, 32), dtype=ml_dtypes.bfloat16)
    return mt, nt, mask, ident, ones, onesb, bg, has_beta


def _get_runner(has_beta=False):
    """Build (once) a sharded jitted executable for the compiled Bass module."""
    key = ("runner", has_beta)
    if key in _CACHE:
        return _CACHE[key]

    import jax
    from jax.sharding import Mesh, PartitionSpec
    from jax.experimental.shard_map import shard_map
    from concourse import bass2jax, mybir

    nckey = ("nc", has_beta)
    nc = _CACHE.get(nckey)
    if nc is None:
        nc = _build(has_beta=has_beta)
        _CACHE[nckey] = nc

    bass2jax.install_neuronx_cc_hook()

    partition_name = nc.partition_id_tensor.name if nc.partition_id_tensor else None
    in_names, out_names, out_avals = [], [], []
    for alloc in nc.m.functions[0].allocations:
        if not isinstance(alloc, mybir.MemoryLocationSet):
            continue
        name = alloc.memorylocations[0].name
        if alloc.kind == "ExternalInput":
            if name != partition_name:
                in_names.append(name)
        elif alloc.kind == "ExternalOutput":
            out_names.append(name)
            shape = tuple(alloc.tensor_shape)
            dtype = mybir.dt.np(alloc.dtype)
            out_avals.append(jax.core.ShapedArray(shape, dtype))
    n_params = len(in_names)
    n_outs = len(out_avals)
    all_in_names = list(in_names) + list(out_names)
    if partition_name is not None:
        all_in_names.append(partition_name)
    donate = tuple(range(n_params, n_params + n_outs))

    def _body(*args):
        operands = list(args)
        if partition_name is not None:
            operands.append(bass2jax.partition_id_tensor())
        outs = bass2jax._bass_exec_p.bind(
            *operands,
            out_avals=tuple(out_avals),
            in_names=tuple(all_in_names),
            out_names=tuple(out_names),
            lowering_input_output_aliases=(),
            sim_require_finite=True,
            sim_require_nnan=True,
            nc=nc,
        )
        return tuple(outs)

    devices = jax.devices()[:8]
    mesh = Mesh(np.asarray(devices), ("core",))
    in_specs = (PartitionSpec("core"),) * (n_params + n_outs)
    out_specs = (PartitionSpec("core"),) * n_outs
    sharded = jax.jit(
        shard_map(_body, mesh=mesh, in_specs=in_specs, out_specs=out_specs,
                  check_rep=False),
        donate_argnums=donate,
        keep_unused=True,
    )
    runner = {
        "sharded": sharded,
        "mesh": mesh,
        "in_names": in_names,
        "out_names": out_names,
        "out_avals": out_avals,
    }
    _CACHE[key] = runner
    return runner


def _run(in_maps, has_beta=False):
    runner = _get_runner(has_beta)
    in_names = runner["in_names"]
    out_avals = runner["out_avals"]
    per_core = [[np.asarray(m[name]) for name in in_names] for m in in_maps]
    concat_in = [
        np.concatenate([per_core[c][i] for c in range(8)], axis=0)
        for i in range(len(in_names))
    ]
    concat_zeros = [
        np.zeros((8 * a.shape[0], *a.shape[1:]), a.dtype) for a in out_avals
    ]
    out_arrs = runner["sharded"](*concat_in, *concat_zeros)
    _CACHE["last_run"] = (concat_in, [tuple(a.shape) for a in out_avals])
    _CACHE["last_beta"] = has_beta
    return [
        {
            name: np.asarray(out_arrs[i]).reshape(8, *out_avals[i].shape)[c]
            for i, name in enumerate(runner["out_names"])
        }
        for c in range(8)
    ]


def bench(iters=10):
    """Time device-side execution with inputs pre-staged on the devices."""
    import time
    import jax
    from jax.sharding import NamedSharding, PartitionSpec

    runner = _get_runner(_CACHE.get("last_beta", False))
    concat_in, out_shapes = _CACHE["last_run"]
    sharding = NamedSharding(runner["mesh"], PartitionSpec("core"))
    dev_in = [jax.device_put(a, sharding) for a in concat_in]
    for a in dev_in:
        a.block_until_ready()
    times = []
    for _ in range(iters):
        zeros = [
            jax.device_put(np.zeros((8 * s[0], *s[1:]), np.float32), sharding)
            for s in out_shapes
        ]
        for z in zeros:
            z.block_until_ready()
        t0 = time.perf_counter()
        outs = runner["sharded"](*dev_in, *zeros)
        for o in outs:
            o.block_until_ready()
        t1 = time.perf_counter()
        times.append(t1 - t0)
    return times


def kernel(x, gamma, beta, wq, wk, wv, wproj):
    x = np.asarray(x, dtype=np.float32)
    gamma = np.asarray(gamma, dtype=np.float32)
    beta = np.asarray(beta, dtype=np.float32)
    wq = np.asarray(wq, dtype=np.float32)
    wk = np.asarray(wk, dtype=np.float32)
    wv = np.asarray(wv, dtype=np.float32)
    wproj = np.asarray(wproj, dtype=np.float32)

    mt, nt, mask, ident, ones, onesb, bg, has_beta = _host_prep(
        gamma, beta, wq, wk, wv, wproj)

    B = x.shape[0]
    in_maps = []
    for i in range(8):
        b, h0 = i // 2, HL * (i % 2)
        in_maps.append({
            "x": np.ascontiguousarray(x[b, :, :, h0:h0 + HL, :]),
            "mt": mt, "nt": nt, "mask": mask, "ident": ident,
            "ones": ones, "onesb": onesb, "bg": bg,
        })

    results = _run(in_maps, has_beta)

    y = np.empty((B, C, T, 2 * HL, W), dtype=np.float32)
    for i in range(8):
        b, h0 = i // 2, HL * (i % 2)
        y[b, :, :, h0:h0 + HL, :] = results[i]["y"]
    return y
